# revision 9
# baseline (speedup 1.0000x reference)
"""KNN classification kernel for Trainium2 (Bass/Tile), 8-core SPMD.

Problem: 1-query KNN over train_data [500000, 256] f32, K=3, 10 classes.
    distances = ||x - train_data||_2  -> top-3 smallest -> mode of targets.

Strategy (fp8 TensorE scan + exact host refinement):
  - Rank by m(t) = 2<x,t> - ||t||^2 (== -d^2 up to the constant ||x||^2).
  - Host precomputes exact fp32 row norms ||t||^2 and ships train_data
    TRANSPOSED in fp8 E4M3 (4x less HBM traffic than f32: 16MB/core).
  - Each core's TensorE computes <x,t> for its 62500 rows: the data tile
    is the STATIONARY operand (lhsT [128 dims x 128 rows], FWL-accelerated
    fp8 weight loads) and x is a 1-column moving operand; each 128-row
    block accumulates a [128,1] psum column over the two 128-dim chunks.
    All 489 columns live in a single PSUM bank [128, 489].
  - DVE epilogue: m = 2*psum - norms, then max_with_indices -> per-
    partition top-8 (value, block) candidates; DMA'd out (8KB).
  - Host merges 8 x 128 x 8 candidates, recomputes EXACT fp64 distances
    for them from the original f32 data (~8K rows), takes the global
    top-3 by (distance, index) and the mode with smallest-on-tie.
    fp8 ranking error (std ~1.1) vs candidate margins (~70) makes the
    top-3 containment rock-solid; the refinement makes the result exact.

Memory-bound target: per-core 16MB fp8 / ~358 GB/s ~= 45us; TensorE
~26-50us of FWL weight loads + FD=1 matmuls, overlapped with DMA.
"""

import sys

import numpy as np

for _p in ("/opt/trn_rl_repo",):
    if _p not in sys.path:
        sys.path.insert(0, _p)

import ml_dtypes

import concourse.bacc as bacc
import concourse.mybir as mybir
from concourse import tile
from concourse.bass_utils import run_bass_kernel_spmd

N_TRAIN = 500000
D = 256
CORES = 8
K = 3
P = 128
NS = N_TRAIN // CORES  # 62500 rows per core
BLOCKS = -(-NS // P)  # 489 psum columns
NSP = BLOCKS * P  # 62592 padded rows per core
RT = 8192  # rows per super-tile (64 blocks); tail tile = 5248 rows
BIG = 1.0e30
FP32 = mybir.dt.float32
F8 = mybir.dt.float8e4
U32 = mybir.dt.uint32
NP_F8 = ml_dtypes.float8_e4m3


def build_knn(tc):
    """Per-core program: fp8 dot products via TensorE + top-8 epilogue."""
    nc = tc.nc
    x_ap = nc.dram_tensor("xq", [P, 2], F8, kind="ExternalInput").ap()
    a_ap = nc.dram_tensor("a", [2 * P, NSP], F8, kind="ExternalInput").ap()
    nrm_ap = nc.dram_tensor("nrm", [P, BLOCKS], FP32, kind="ExternalInput").ap()
    vals_ap = nc.dram_tensor("out_vals", [P, 8], FP32, kind="ExternalOutput").ap()
    idx_ap = nc.dram_tensor("out_idx", [P, 8], U32, kind="ExternalOutput").ap()

    with (
        tc.tile_pool(name="xp", bufs=1) as x_pool,
        tc.tile_pool(name="np", bufs=1) as n_pool,
        tc.tile_pool(name="inp", bufs=1) as in_pool,
        tc.tile_pool(name="psp", bufs=1, space="PSUM") as ps_pool,
        tc.tile_pool(name="outp", bufs=1) as out_pool,
    ):
        # norms first: they ride along early and can't straggle into the
        # epilogue's critical path
        nrm_t = n_pool.tile([P, BLOCKS], FP32)
        nc.scalar.dma_start(out=nrm_t[:], in_=nrm_ap[:, :])
        x_t = x_pool.tile([P, 2], F8)
        nc.sync.dma_start(out=x_t[:], in_=x_ap[:, :])

        ps = ps_pool.tile([P, BLOCKS], FP32)

        # Descending tile sizes: big transfers amortize issue cost up front,
        # many small ones at the end keep all DMA engines busy to the last
        # byte. Chunk 0 issues on the SP HWDGE queue, chunk 1 on the ACT
        # HWDGE queue (two parallel issue streams).
        blocks_per_tile = [64, 64, 64, 48, 48, 32, 32, 24, 24, 16, 16, 12, 12, 8, 8, 6, 5, 4, 2]
        assert sum(blocks_per_tile) == BLOCKS

        col = 0
        r = 0
        for ti, nb in enumerate(blocks_per_tile):
            rt = nb * P
            t0 = in_pool.tile([P, rt], F8, tag=f"c0_{ti}")
            t1 = in_pool.tile([P, rt], F8, tag=f"c1_{ti}")
            nc.sync.dma_start(out=t0[:, 0:rt], in_=a_ap[0:P, r : r + rt])
            nc.scalar.dma_start(out=t1[:, 0:rt], in_=a_ap[P : 2 * P, r : r + rt])
            for b in range(rt // P):
                nc.tensor.matmul(
                    ps[:, col : col + 1],
                    lhsT=t0[:, b * P : (b + 1) * P],
                    rhs=x_t[:, 0:1],
                    start=True,
                    stop=False,
                )
                nc.tensor.matmul(
                    ps[:, col : col + 1],
                    lhsT=t1[:, b * P : (b + 1) * P],
                    rhs=x_t[:, 1:2],
                    start=False,
                    stop=True,
                )
                col += 1
            r += rt
        assert col == BLOCKS and r == NSP

        # m = 2*dot - ||t||^2  (= -d^2 + const; maximize)
        m_t = out_pool.tile([P, BLOCKS], FP32)
        nc.vector.scalar_tensor_tensor(
            out=m_t[:],
            in0=ps[:],
            scalar=2.0,
            in1=nrm_t[:],
            op0=mybir.AluOpType.mult,
            op1=mybir.AluOpType.subtract,
        )
        valt = out_pool.tile([P, 8], FP32)
        idxt = out_pool.tile([P, 8], U32)
        nc.vector.max_with_indices(valt[:], idxt[:], m_t[:])
        nc.sync.dma_start(out=vals_ap[:, :], in_=valt[:])
        nc.sync.dma_start(out=idx_ap[:, :], in_=idxt[:])


_PROGRAM_CACHE = {}


def get_program():
    if "nc" not in _PROGRAM_CACHE:
        nc = bacc.Bacc(
            "TRN2", target_bir_lowering=False, debug=False, num_devices=CORES
        )
        with tile.TileContext(nc) as tc:
            build_knn(tc)
        nc.compile()
        _PROGRAM_CACHE["nc"] = nc
    return _PROGRAM_CACHE["nc"]


def run_device(in_maps, trace=False, trace_cores=None):
    nc = get_program()
    return run_bass_kernel_spmd(
        nc, in_maps, list(range(CORES)), trace=trace, trace_cores=trace_cores
    )


def make_in_maps(x, train_data):
    x = np.asarray(x, dtype=np.float32)
    train_data = np.asarray(train_data, dtype=np.float32)
    td8 = train_data.astype(NP_F8)
    x8 = x.astype(NP_F8)
    xq = np.ascontiguousarray(x8.reshape(2, P).T)  # [128, 2]
    norms = np.einsum("nd,nd->n", train_data, train_data, dtype=np.float64)
    norms = norms.astype(np.float32)
    in_maps = []
    for c in range(CORES):
        a = np.zeros((2 * P, NSP), dtype=NP_F8)
        a[:, :NS] = td8[c * NS : (c + 1) * NS].T
        nrm = np.full(NSP, BIG, dtype=np.float32)
        nrm[:NS] = norms[c * NS : (c + 1) * NS]
        nrm = np.ascontiguousarray(nrm.reshape(BLOCKS, P).T)  # [128, BLOCKS]
        in_maps.append({"xq": xq, "a": a, "nrm": nrm})
    return in_maps


def merge_results(results, x, train_data, train_targets):
    """Merge per-core candidates; refine with exact distances on host."""
    x64 = np.asarray(x, dtype=np.float64)
    td = np.asarray(train_data)
    p_idx = np.arange(P, dtype=np.int64)[:, None]
    cands = []
    for c in range(len(results)):
        v = np.asarray(results[c]["out_vals"], dtype=np.float64)
        ix = np.asarray(results[c]["out_idx"], dtype=np.int64)
        rl = ix * P + p_idx  # core-local row
        valid = (v > -BIG / 2) & (rl < NS)
        cands.append((c * NS + rl)[valid])
    g = np.unique(np.concatenate(cands))
    d2 = ((td[g].astype(np.float64) - x64) ** 2).sum(axis=1)
    order = np.lexsort((g, d2))  # distance asc, then index asc (top_k ties)
    top = g[order[:K]]
    knn_t = np.asarray(train_targets)[top]
    # torch .mode(): most frequent value, smallest value on ties
    counts = (knn_t[:, None] == knn_t[None, :]).sum(axis=1)
    sentinel = np.iinfo(knn_t.dtype).max
    cands_cls = np.where(counts == counts.max(), knn_t, sentinel)
    return cands_cls.min()


def kernel(x, train_data, train_targets):
    train_targets = np.asarray(train_targets)
    in_maps = make_in_maps(x, train_data)
    results = run_device(in_maps).results
    pred = merge_results(results, x, train_data, train_targets)
    return np.array(pred, dtype=train_targets.dtype)


# revision 11
# speedup vs baseline: 1.0693x; 1.0693x over previous
"""KNN classification kernel for Trainium2 (Bass/Tile), 8-core SPMD.

Problem: 1-query KNN over train_data [500000, 256] f32, K=3, 10 classes.
    distances = ||x - train_data||_2  -> top-3 smallest -> mode of targets.

Strategy (fp8 TensorE scan + exact host refinement):
  - Rank by m(t) = 2<x,t> - ||t||^2 (== -d^2 up to the constant ||x||^2).
  - Host precomputes exact fp32 row norms ||t||^2 and ships train_data
    TRANSPOSED in fp8 E4M3 (4x less HBM traffic than f32: 16MB/core).
  - Each core's TensorE computes <x,t> for its 62500 rows: the data tile
    is the STATIONARY operand (lhsT [128 dims x 128 rows], FWL-accelerated
    fp8 weight loads) and x is a 1-column moving operand; each 128-row
    block accumulates a [128,1] psum column over the two 128-dim chunks.
    All 489 columns live in a single PSUM bank [128, 489].
  - DVE epilogue: m = 2*psum - norms, then max_with_indices -> per-
    partition top-8 (value, block) candidates; DMA'd out (8KB).
  - Host merges 8 x 128 x 8 candidates, recomputes EXACT fp64 distances
    for them from the original f32 data (~8K rows), takes the global
    top-3 by (distance, index) and the mode with smallest-on-tie.
    fp8 ranking error (std ~1.1) vs candidate margins (~70) makes the
    top-3 containment rock-solid; the refinement makes the result exact.

Memory-bound target: per-core 16MB fp8 / ~358 GB/s ~= 45us; TensorE
~26-50us of FWL weight loads + FD=1 matmuls, overlapped with DMA.
"""

import sys

import numpy as np

for _p in ("/opt/trn_rl_repo",):
    if _p not in sys.path:
        sys.path.insert(0, _p)

import ml_dtypes

import concourse.bacc as bacc
import concourse.mybir as mybir
from concourse import tile
from concourse.bass_utils import run_bass_kernel_spmd

N_TRAIN = 500000
D = 256
CORES = 8
K = 3
P = 128
NS = N_TRAIN // CORES  # 62500 rows per core
BLOCKS = -(-NS // P)  # 489 psum columns
NSP = BLOCKS * P  # 62592 padded rows per core
RT = 8192  # rows per super-tile (64 blocks); tail tile = 5248 rows
BIG = 1.0e30
FP32 = mybir.dt.float32
F8 = mybir.dt.float8e4
U32 = mybir.dt.uint32
NP_F8 = ml_dtypes.float8_e4m3


def build_knn(tc):
    """Per-core program: fp8 dot products via TensorE + top-8 epilogue."""
    nc = tc.nc
    x_ap = nc.dram_tensor("xq", [P, 2], F8, kind="ExternalInput").ap()
    a_ap = nc.dram_tensor("a", [2 * P, NSP], F8, kind="ExternalInput").ap()
    nrm_ap = nc.dram_tensor("nrm", [P, BLOCKS], FP32, kind="ExternalInput").ap()
    vals_ap = nc.dram_tensor("out_vals", [P, 8], FP32, kind="ExternalOutput").ap()
    idx_ap = nc.dram_tensor("out_idx", [P, 8], U32, kind="ExternalOutput").ap()

    with (
        tc.tile_pool(name="xp", bufs=1) as x_pool,
        tc.tile_pool(name="np", bufs=1) as n_pool,
        tc.tile_pool(name="inp", bufs=8) as in_pool,
        tc.tile_pool(name="psp", bufs=1, space="PSUM") as ps_pool,
        tc.tile_pool(name="outp", bufs=1) as out_pool,
    ):
        # norms first: they ride along early and can't straggle into the
        # epilogue's critical path
        nrm_t = n_pool.tile([P, BLOCKS], FP32)
        nc.scalar.dma_start(out=nrm_t[:], in_=nrm_ap[:, :])
        x_t = x_pool.tile([P, 2], F8)
        nc.sync.dma_start(out=x_t[:], in_=x_ap[:, :])

        ps = ps_pool.tile([P, BLOCKS], FP32)

        # Uniform ~256KB transfers: the Tile scheduler paces DMA issue on an
        # ~8-lane completion window and each DMA drains at ~2 engines, so
        # equal mid-size DMAs keep all engines fed and the tail short.
        # Chunk 0 issues on the SP HWDGE queue, chunk 1 on the ACT queue.
        blocks_per_tile = [16] * 30 + [9]
        assert sum(blocks_per_tile) == BLOCKS

        col = 0
        r = 0
        for nb in blocks_per_tile:
            rt = nb * P
            t0 = in_pool.tile([P, 16 * P], F8, tag="c0")
            t1 = in_pool.tile([P, 16 * P], F8, tag="c1")
            nc.sync.dma_start(out=t0[:, 0:rt], in_=a_ap[0:P, r : r + rt])
            nc.scalar.dma_start(out=t1[:, 0:rt], in_=a_ap[P : 2 * P, r : r + rt])
            for b in range(rt // P):
                nc.tensor.matmul(
                    ps[:, col : col + 1],
                    lhsT=t0[:, b * P : (b + 1) * P],
                    rhs=x_t[:, 0:1],
                    start=True,
                    stop=False,
                )
                nc.tensor.matmul(
                    ps[:, col : col + 1],
                    lhsT=t1[:, b * P : (b + 1) * P],
                    rhs=x_t[:, 1:2],
                    start=False,
                    stop=True,
                )
                col += 1
            r += rt
        assert col == BLOCKS and r == NSP

        # m = 2*dot - ||t||^2  (= -d^2 + const; maximize)
        m_t = out_pool.tile([P, BLOCKS], FP32)
        nc.vector.scalar_tensor_tensor(
            out=m_t[:],
            in0=ps[:],
            scalar=2.0,
            in1=nrm_t[:],
            op0=mybir.AluOpType.mult,
            op1=mybir.AluOpType.subtract,
        )
        valt = out_pool.tile([P, 8], FP32)
        idxt = out_pool.tile([P, 8], U32)
        nc.vector.max_with_indices(valt[:], idxt[:], m_t[:])
        nc.sync.dma_start(out=vals_ap[:, :], in_=valt[:])
        nc.sync.dma_start(out=idx_ap[:, :], in_=idxt[:])


_PROGRAM_CACHE = {}


def get_program():
    if "nc" not in _PROGRAM_CACHE:
        nc = bacc.Bacc(
            "TRN2", target_bir_lowering=False, debug=False, num_devices=CORES
        )
        with tile.TileContext(nc) as tc:
            build_knn(tc)
        nc.compile()
        _PROGRAM_CACHE["nc"] = nc
    return _PROGRAM_CACHE["nc"]


def run_device(in_maps, trace=False, trace_cores=None):
    nc = get_program()
    return run_bass_kernel_spmd(
        nc, in_maps, list(range(CORES)), trace=trace, trace_cores=trace_cores
    )


def make_in_maps(x, train_data):
    x = np.asarray(x, dtype=np.float32)
    train_data = np.asarray(train_data, dtype=np.float32)
    td8 = train_data.astype(NP_F8)
    x8 = x.astype(NP_F8)
    xq = np.ascontiguousarray(x8.reshape(2, P).T)  # [128, 2]
    norms = np.einsum("nd,nd->n", train_data, train_data, dtype=np.float64)
    norms = norms.astype(np.float32)
    in_maps = []
    for c in range(CORES):
        a = np.zeros((2 * P, NSP), dtype=NP_F8)
        a[:, :NS] = td8[c * NS : (c + 1) * NS].T
        nrm = np.full(NSP, BIG, dtype=np.float32)
        nrm[:NS] = norms[c * NS : (c + 1) * NS]
        nrm = np.ascontiguousarray(nrm.reshape(BLOCKS, P).T)  # [128, BLOCKS]
        in_maps.append({"xq": xq, "a": a, "nrm": nrm})
    return in_maps


def merge_results(results, x, train_data, train_targets):
    """Merge per-core candidates; refine with exact distances on host."""
    x64 = np.asarray(x, dtype=np.float64)
    td = np.asarray(train_data)
    p_idx = np.arange(P, dtype=np.int64)[:, None]
    cands = []
    for c in range(len(results)):
        v = np.asarray(results[c]["out_vals"], dtype=np.float64)
        ix = np.asarray(results[c]["out_idx"], dtype=np.int64)
        rl = ix * P + p_idx  # core-local row
        valid = (v > -BIG / 2) & (rl < NS)
        cands.append((c * NS + rl)[valid])
    g = np.unique(np.concatenate(cands))
    d2 = ((td[g].astype(np.float64) - x64) ** 2).sum(axis=1)
    order = np.lexsort((g, d2))  # distance asc, then index asc (top_k ties)
    top = g[order[:K]]
    knn_t = np.asarray(train_targets)[top]
    # torch .mode(): most frequent value, smallest value on ties
    counts = (knn_t[:, None] == knn_t[None, :]).sum(axis=1)
    sentinel = np.iinfo(knn_t.dtype).max
    cands_cls = np.where(counts == counts.max(), knn_t, sentinel)
    return cands_cls.min()


def kernel(x, train_data, train_targets):
    train_targets = np.asarray(train_targets)
    in_maps = make_in_maps(x, train_data)
    results = run_device(in_maps).results
    pred = merge_results(results, x, train_data, train_targets)
    return np.array(pred, dtype=train_targets.dtype)


# revision 13
# speedup vs baseline: 1.0849x; 1.0146x over previous
"""KNN classification kernel for Trainium2 (Bass/Tile), 8-core SPMD.

Problem: 1-query KNN over train_data [500000, 256] f32, K=3, 10 classes.
    distances = ||x - train_data||_2  -> top-3 smallest -> mode of targets.

Strategy (fp8 TensorE scan + exact host refinement):
  - Rank by m(t) = 2<x,t> - ||t||^2 (== -d^2 up to the constant ||x||^2).
  - Host precomputes exact fp32 row norms ||t||^2 and ships train_data
    TRANSPOSED in fp8 E4M3 (4x less HBM traffic than f32: 16MB/core).
  - Each core's TensorE computes <x,t> for its 62500 rows: the data tile
    is the STATIONARY operand (lhsT [128 dims x 128 rows], FWL-accelerated
    fp8 weight loads) and x is a 1-column moving operand; each 128-row
    block accumulates a [128,1] psum column over the two 128-dim chunks.
    All 489 columns live in a single PSUM bank [128, 489].
  - DVE epilogue: m = 2*psum - norms, then max_with_indices -> per-
    partition top-8 (value, block) candidates; DMA'd out (8KB).
  - Host merges 8 x 128 x 8 candidates, recomputes EXACT fp64 distances
    for them from the original f32 data (~8K rows), takes the global
    top-3 by (distance, index) and the mode with smallest-on-tie.
    fp8 ranking error (std ~1.1) vs candidate margins (~70) makes the
    top-3 containment rock-solid; the refinement makes the result exact.

Memory-bound target: per-core 16MB fp8 / ~358 GB/s ~= 45us; TensorE
~26-50us of FWL weight loads + FD=1 matmuls, overlapped with DMA.
"""

import sys

import numpy as np

for _p in ("/opt/trn_rl_repo",):
    if _p not in sys.path:
        sys.path.insert(0, _p)

import ml_dtypes

import concourse.bacc as bacc
import concourse.mybir as mybir
from concourse import tile
from concourse.bass_utils import run_bass_kernel_spmd

N_TRAIN = 500000
D = 256
CORES = 8
K = 3
P = 128
NS = N_TRAIN // CORES  # 62500 rows per core
BLOCKS = -(-NS // P)  # 489 psum columns
NSP = BLOCKS * P  # 62592 padded rows per core
RT = 8192  # rows per super-tile (64 blocks); tail tile = 5248 rows
BIG = 1.0e30
FP32 = mybir.dt.float32
F8 = mybir.dt.float8e4
U32 = mybir.dt.uint32
NP_F8 = ml_dtypes.float8_e4m3


def build_knn(tc):
    """Per-core program: fp8 dot products via TensorE + top-8 epilogue."""
    nc = tc.nc
    x_ap = nc.dram_tensor("xq", [P, 2], F8, kind="ExternalInput").ap()
    a_ap = nc.dram_tensor("a", [2 * P, NSP], F8, kind="ExternalInput").ap()
    nrm_ap = nc.dram_tensor("nrm", [P, BLOCKS], FP32, kind="ExternalInput").ap()
    vals_ap = nc.dram_tensor("out_vals", [P, 8], FP32, kind="ExternalOutput").ap()
    idx_ap = nc.dram_tensor("out_idx", [P, 8], U32, kind="ExternalOutput").ap()

    with (
        tc.tile_pool(name="xp", bufs=1) as x_pool,
        tc.tile_pool(name="np", bufs=1) as n_pool,
        tc.tile_pool(name="inp", bufs=1) as in_pool,
        tc.tile_pool(name="psp", bufs=1, space="PSUM") as ps_pool,
        tc.tile_pool(name="outp", bufs=1) as out_pool,
    ):
        # norms first: they ride along early and can't straggle into the
        # epilogue's critical path
        nrm_t = n_pool.tile([P, BLOCKS], FP32)
        nc.scalar.dma_start(out=nrm_t[:], in_=nrm_ap[:, :])
        x_t = x_pool.tile([P, 2], F8)
        nc.sync.dma_start(out=x_t[:], in_=x_ap[:, :])

        # Two psum tiles so the first half's epilogue STT can overlap the
        # second half's matmuls.
        SPLIT = 256
        ps_a = ps_pool.tile([P, SPLIT], FP32)
        ps_b = ps_pool.tile([P, BLOCKS - SPLIT], FP32)
        m_t = out_pool.tile([P, BLOCKS], FP32)

        # Uniform big transfers burst-issued up front (deep buffer pool), a
        # few smaller ones at the end to shorten the last-transfer drain.
        # Chunk 0 issues on the SP HWDGE queue, chunk 1 on the ACT queue.
        blocks_per_tile = [64] * 6 + [32, 32, 16, 16, 9]
        assert sum(blocks_per_tile) == BLOCKS

        def do_epilogue_half(lo, hi, ps):
            nc.vector.scalar_tensor_tensor(
                out=m_t[:, lo:hi],
                in0=ps[:],
                scalar=2.0,
                in1=nrm_t[:, lo:hi],
                op0=mybir.AluOpType.mult,
                op1=mybir.AluOpType.subtract,
            )

        col = 0
        r = 0
        for ti, nb in enumerate(blocks_per_tile):
            rt = nb * P
            t0 = in_pool.tile([P, rt], F8, tag=f"c0_{ti}")
            t1 = in_pool.tile([P, rt], F8, tag=f"c1_{ti}")
            nc.sync.dma_start(out=t0[:, 0:rt], in_=a_ap[0:P, r : r + rt])
            nc.scalar.dma_start(out=t1[:, 0:rt], in_=a_ap[P : 2 * P, r : r + rt])
            for b in range(rt // P):
                if col < SPLIT:
                    pcol, ps = col, ps_a
                else:
                    pcol, ps = col - SPLIT, ps_b
                nc.tensor.matmul(
                    ps[:, pcol : pcol + 1],
                    lhsT=t0[:, b * P : (b + 1) * P],
                    rhs=x_t[:, 0:1],
                    start=True,
                    stop=False,
                )
                nc.tensor.matmul(
                    ps[:, pcol : pcol + 1],
                    lhsT=t1[:, b * P : (b + 1) * P],
                    rhs=x_t[:, 1:2],
                    start=False,
                    stop=True,
                )
                col += 1
                if col == SPLIT:
                    # first half of m = 2*dot - ||t||^2 while PE continues
                    do_epilogue_half(0, SPLIT, ps_a)
            r += rt
        assert col == BLOCKS and r == NSP

        do_epilogue_half(SPLIT, BLOCKS, ps_b)
        valt = out_pool.tile([P, 8], FP32)
        idxt = out_pool.tile([P, 8], U32)
        nc.vector.max_with_indices(valt[:], idxt[:], m_t[:])
        nc.sync.dma_start(out=vals_ap[:, :], in_=valt[:])
        nc.sync.dma_start(out=idx_ap[:, :], in_=idxt[:])


_PROGRAM_CACHE = {}


def get_program():
    if "nc" not in _PROGRAM_CACHE:
        nc = bacc.Bacc(
            "TRN2", target_bir_lowering=False, debug=False, num_devices=CORES
        )
        with tile.TileContext(nc) as tc:
            build_knn(tc)
        nc.compile()
        _PROGRAM_CACHE["nc"] = nc
    return _PROGRAM_CACHE["nc"]


def run_device(in_maps, trace=False, trace_cores=None):
    nc = get_program()
    return run_bass_kernel_spmd(
        nc, in_maps, list(range(CORES)), trace=trace, trace_cores=trace_cores
    )


def make_in_maps(x, train_data):
    x = np.asarray(x, dtype=np.float32)
    train_data = np.asarray(train_data, dtype=np.float32)
    td8 = train_data.astype(NP_F8)
    x8 = x.astype(NP_F8)
    xq = np.ascontiguousarray(x8.reshape(2, P).T)  # [128, 2]
    norms = np.einsum("nd,nd->n", train_data, train_data, dtype=np.float64)
    norms = norms.astype(np.float32)
    in_maps = []
    for c in range(CORES):
        a = np.zeros((2 * P, NSP), dtype=NP_F8)
        a[:, :NS] = td8[c * NS : (c + 1) * NS].T
        nrm = np.full(NSP, BIG, dtype=np.float32)
        nrm[:NS] = norms[c * NS : (c + 1) * NS]
        nrm = np.ascontiguousarray(nrm.reshape(BLOCKS, P).T)  # [128, BLOCKS]
        in_maps.append({"xq": xq, "a": a, "nrm": nrm})
    return in_maps


def merge_results(results, x, train_data, train_targets):
    """Merge per-core candidates; refine with exact distances on host."""
    x64 = np.asarray(x, dtype=np.float64)
    td = np.asarray(train_data)
    p_idx = np.arange(P, dtype=np.int64)[:, None]
    cands = []
    for c in range(len(results)):
        v = np.asarray(results[c]["out_vals"], dtype=np.float64)
        ix = np.asarray(results[c]["out_idx"], dtype=np.int64)
        rl = ix * P + p_idx  # core-local row
        valid = (v > -BIG / 2) & (rl < NS)
        cands.append((c * NS + rl)[valid])
    g = np.unique(np.concatenate(cands))
    d2 = ((td[g].astype(np.float64) - x64) ** 2).sum(axis=1)
    order = np.lexsort((g, d2))  # distance asc, then index asc (top_k ties)
    top = g[order[:K]]
    knn_t = np.asarray(train_targets)[top]
    # torch .mode(): most frequent value, smallest value on ties
    counts = (knn_t[:, None] == knn_t[None, :]).sum(axis=1)
    sentinel = np.iinfo(knn_t.dtype).max
    cands_cls = np.where(counts == counts.max(), knn_t, sentinel)
    return cands_cls.min()


def kernel(x, train_data, train_targets):
    train_targets = np.asarray(train_targets)
    in_maps = make_in_maps(x, train_data)
    results = run_device(in_maps).results
    pred = merge_results(results, x, train_data, train_targets)
    return np.array(pred, dtype=train_targets.dtype)
